# revision 11
# baseline (speedup 1.0000x reference)
"""Trainium2 Bass kernel for nn_AdaGMNConv (gnn_message_passing).

Sharding: one graph per NeuronCore (G=8 graphs, 8 cores). All compute is
local to a core; the host gathers the per-graph scalar outputs.

Per-core math (graph g, M=2048 high-degree nodes per graph, D=128):
  A    = H_g @ F^T                      [2048, 2048]   (bf16 matmul, f32 psum)
  A1   = segment softmax of A over rows (per column)   -> S1 = A1 @ F
  A2   = softmax of A over columns (per row)           -> S2 = A2^T @ H
  out_multi  = MLP([H | S1]); out_single = MLP([F | S2])
  p2 = colsum(out_multi) + colsum(L_g);  p1 = colsum(out_single) + colsum(L_gid)
  out[g] = <p1/||p1||, p2/||p2||>

Schedule (single exp pass; E^T via DMA transpose):
  P1 (per j-tile t): A^T-chunk MMs -> fused exp(+row-sum accum = den1)
     -> F~ = F/den1 -> S1T accumulation MMs; DMA-transpose E1_t into E2.
  P2: S1T evac; den2 = col-sums of E1 via ones-matmuls (chunked, PSUM
     rotating), scatter to per-partition layout, H~ = H/den2 -> S2T MMs;
     MLP(out_multi) interleaved on ACT/DVE.
  P3: S2T evac; MLP(out_single); pooled vectors; cosine output.

MLP tricks: W1 is host-augmented with a 129th column = W1 @ 1/128 so the
row-mean of the pre-activation falls out of the matmul; LayerNorm+ReLU is
one ScalarE op (scale=rstd, bias=-mu*rstd per partition); the column sum
of the ReLU output accumulates through 1-wide matmuls. L/L0 column sums
are host-precomputed (they are linear in x).
"""

import os
from contextlib import ExitStack

import numpy as np

import concourse.bass as bass
import concourse.tile as tile
from concourse import mybir
from concourse.bass_utils import run_bass_kernel_spmd

f32 = mybir.dt.float32
bf16 = mybir.dt.bfloat16

P = 128          # partitions
D = 128          # feature dim
NT = 16          # tiles per 2048-node block
M = P * NT       # 2048 nodes per block
SHIFT0 = 64.0    # exp shift for non-gid cores
LN_EPS = 1e-5
CH = 1024        # PSUM chunk width for the A tiles (2 banks)
MMN = 512        # matmul moving free-dim max

MAXW = 1  # walrus in this env rejects >1 sem-wait per instruction


def split_waits(nc, maxw=MAXW):
    """Hoist overflow sem-waits onto preceding same-engine NOPs (this walrus
    build only accepts `maxw` waits per instruction)."""
    ctr = 0
    for fn in nc.m.functions:
        for bb in fn.blocks:
            new_insts = []
            for inst in bb.instructions:
                si = inst.sync_info
                if si is not None and si.on_wait and len(si.on_wait) > maxw:
                    waits = list(si.on_wait)
                    chunks = [waits[i : i + maxw] for i in range(0, len(waits), maxw)]
                    for ch in chunks[:-1]:
                        ctr += 1
                        nop = mybir.InstNoOp(
                            name=f"waitsplit_{ctr}",
                            sync_info=mybir.SyncInfo(on_wait=ch, on_update=[]),
                            bass_nofuse=True,
                            engine=inst.engine,
                        )
                        new_insts.append(nop)
                    si.on_wait = chunks[-1]
                new_insts.append(inst)
            bb.instructions = new_insts
    return ctr


def build_nc(has_b1, has_b2, has_gamma, has_beta):
    nc = bass.Bass()
    WA = 132  # padded aug width (129 used)

    # ---- DRAM parameters (per-core shard shapes, host-packed layouts) ----
    dHTb = nc.declare_dram_parameter("HTb", [P, M], bf16, isOutput=False)
    dFTb = nc.declare_dram_parameter("FTb", [P, M], bf16, isOutput=False)
    dHb = nc.declare_dram_parameter("Hb", [P, M], bf16, isOutput=False)
    dFb = nc.declare_dram_parameter("Fb", [P, M], bf16, isOutput=False)
    dW1 = nc.declare_dram_parameter("W1a", [P, 2, WA], bf16, isOutput=False)
    dW2 = nc.declare_dram_parameter("W2", [P, D], f32, isOutput=False)
    dNEG = nc.declare_dram_parameter("NEG", [P, 1], f32, isOutput=False)
    dLV = nc.declare_dram_parameter("LV", [P, 2], f32, isOutput=False)
    dB1 = dB2 = dGAM = dBET = None
    if has_b1:
        dB1 = nc.declare_dram_parameter("B1a", [1, WA], bf16, isOutput=False)
    if has_b2:
        dB2 = nc.declare_dram_parameter("B2", [1, D], f32, isOutput=False)
    if has_gamma:
        dGAM = nc.declare_dram_parameter("GAM", [1, D], f32, isOutput=False)
    if has_beta:
        dBET = nc.declare_dram_parameter("BET", [1, D], f32, isOutput=False)
    dOUT = nc.declare_dram_parameter("out", [1, 1], f32, isOutput=True)

    gen_ln = has_gamma or has_beta  # generic LN path on DVE

    with tile.TileContext(nc) as tc, ExitStack() as ctx:
        consts = ctx.enter_context(tc.tile_pool(name="consts", bufs=1))
        scal = ctx.enter_context(tc.tile_pool(name="scal", bufs=6))
        fpp = ctx.enter_context(tc.tile_pool(name="fpp", bufs=3))
        mlpt = ctx.enter_context(tc.tile_pool(name="mlpt", bufs=3))

        # ---- SBUF input loads (fat contiguous DMAs, host-packed) ----
        sb_FTb = consts.tile([P, M], bf16)
        nc.sync.dma_start(out=sb_FTb, in_=dFTb[:, :])
        sb_HTb = consts.tile([P, M], bf16)
        for c in range(2):
            nc.sync.dma_start(out=sb_HTb[:, bass.ts(c, M // 2)],
                              in_=dHTb[:, bass.ts(c, M // 2)])
        sb_NEG = consts.tile([P, 1], f32)
        nc.scalar.dma_start(out=sb_NEG, in_=dNEG[:, :])
        sb_Fb = consts.tile([P, M], bf16)
        nc.scalar.dma_start(out=sb_Fb, in_=dFb[:, :])
        sb_Hb = consts.tile([P, M], bf16)
        nc.gpsimd.dma_start(out=sb_Hb, in_=dHb[:, :])
        sb_W1 = consts.tile([P, 2, WA], bf16)
        nc.gpsimd.dma_start(out=sb_W1, in_=dW1[:, :, :])
        sb_W2 = consts.tile([P, D], f32)
        nc.gpsimd.dma_start(out=sb_W2, in_=dW2[:, :])
        sb_LV = consts.tile([P, 2], f32)
        nc.gpsimd.dma_start(out=sb_LV, in_=dLV[:, :])

        sb_B1 = sb_B2 = None
        if has_b1:
            sb_B1 = consts.tile([1, WA], bf16)
            nc.gpsimd.dma_start(out=sb_B1, in_=dB1[:, :])
        if has_b2:
            sb_B2 = consts.tile([1, D], f32)
            nc.gpsimd.dma_start(out=sb_B2, in_=dB2[:, :])
        gam_bc = bet_bc = None
        if has_gamma:
            gam_bc = consts.tile([P, D], f32)
            src = dGAM[:, :]
            nc.gpsimd.dma_start(
                out=gam_bc,
                in_=bass.AP(tensor=src.tensor, offset=src.offset,
                            ap=[[0, P], src.ap[1]]),
            )
        if has_beta:
            bet_bc = consts.tile([P, D], f32)
            src = dBET[:, :]
            nc.gpsimd.dma_start(
                out=bet_bc,
                in_=bass.AP(tensor=src.tensor, offset=src.offset,
                            ap=[[0, P], src.ap[1]]),
            )

        ones_f = consts.tile([P, 1], f32)
        nc.vector.memset(ones_f, 1.0)
        ones_b = consts.tile([P, 1], bf16)
        nc.vector.memset(ones_b, 1.0)
        ones_row = consts.tile([1, D], bf16)
        nc.vector.memset(ones_row, 1.0)
        sb_eps = consts.tile([P, 1], f32)
        nc.vector.memset(sb_eps, LN_EPS)
        c2048 = consts.tile([1, 1], f32)
        nc.vector.memset(c2048, float(M))

        sb_E1 = consts.tile([P, NT, M], bf16)
        sb_E2 = consts.tile([P, NT, M], bf16)
        sb_S1Tb = consts.tile([P, M], bf16)
        sb_S2Tb = consts.tile([P, M], bf16)
        den2p = consts.tile([P, NT], f32)
        rec2 = consts.tile([P, NT], f32)
        sb_d2row = consts.tile([1, M], f32)

        # =========== P1: A^T tiles -> exp/den1 -> F~ -> S1T; transpose ======
        with tc.tile_pool(name="psS", bufs=1, space="PSUM") as psS, \
             tc.tile_pool(name="psA", bufs=2, space="PSUM") as psA:
            ps_s1t = psS.tile([P, M], f32, tag="psS")
            for t in range(NT):
                tr = bass.ts(t, P)
                dp = scal.tile([P, 2], f32, tag="dp")
                for c in range(M // CH):
                    pa = psA.tile([P, CH], f32, tag="psA")
                    for h in range(CH // MMN):
                        ci = c * (CH // MMN) + h
                        nc.tensor.matmul(
                            pa[:, bass.ts(h, MMN)], lhsT=sb_FTb[:, tr],
                            rhs=sb_HTb[:, bass.ts(ci, MMN)],
                            start=True, stop=True,
                        )
                    nc.scalar.activation(
                        out=sb_E1[:, t, bass.ts(c, CH)], in_=pa,
                        func=mybir.ActivationFunctionType.Exp,
                        bias=sb_NEG, scale=1.0,
                        accum_out=dp[:, c : c + 1],
                    )
                den = scal.tile([P, 1], f32, tag="den")
                nc.vector.reduce_sum(out=den, in_=dp, axis=mybir.AxisListType.X)
                rec = scal.tile([P, 1], f32, tag="rec")
                nc.vector.reciprocal(out=rec, in_=den)
                fp = fpp.tile([P, D], bf16, tag="fp")
                nc.vector.tensor_scalar_mul(out=fp, in0=sb_Fb[:, tr], scalar1=rec)
                for h in range(M // MMN):
                    hs = bass.ts(h, MMN)
                    nc.tensor.matmul(
                        ps_s1t[:, hs], lhsT=fp, rhs=sb_E1[:, t, hs],
                        start=(t == 0), stop=(t == NT - 1),
                    )
                nc.sync.dma_start_transpose(out=sb_E2[:, :, tr], in_=sb_E1[:, t, :])

            # S1T evac (split DVE/ACT halves to spread the cost)
            nc.vector.tensor_copy(out=sb_S1Tb[:, 0:CH], in_=ps_s1t[:, 0:CH])
            nc.scalar.copy(out=sb_S1Tb[:, CH:M], in_=ps_s1t[:, CH:M])

        # ============= P2: den2 (ones-MMs) -> H~ -> S2T; MLP2 ==============
        def mlp_tile(t, xTb, sTb, ps_r, first, last, pool_ps, pool_sb):
            """One MLP tile: pre-act (+mean col), LN+ReLU, colsum accumulate."""
            tr = bass.ts(t, P)
            pre = pool_ps.tile([P, WA], f32, tag="pre")
            nc.tensor.matmul(pre[:, 0:129], lhsT=xTb[:, tr], rhs=sb_W1[:, 0, 0:129],
                             start=True, stop=False)
            nc.tensor.matmul(pre[:, 0:129], lhsT=sTb[:, tr], rhs=sb_W1[:, 1, 0:129],
                             start=False, stop=not has_b1)
            if has_b1:
                nc.tensor.matmul(pre[:, 0:129], lhsT=ones_row, rhs=sb_B1[:, 0:129],
                                 start=False, stop=True)
            mu = pre[:, 128:129]
            # ssq = sum_d (pre - mu)^2 via ACT Square with bias=-mu
            nmu = scal.tile([P, 1], f32, tag="nmu")
            nc.vector.tensor_scalar_mul(out=nmu, in0=mu, scalar1=-1.0)
            trash = pool_sb.tile([P, D], bf16, tag="trash")
            ssq = scal.tile([P, 1], f32, tag="ssq")
            nc.scalar.activation(out=trash, in_=pre[:, 0:D],
                                 func=mybir.ActivationFunctionType.Square,
                                 bias=nmu, scale=1.0, accum_out=ssq)
            # rstd = exp(-0.5 * ln(ssq/128 + eps))
            lnv = scal.tile([P, 1], f32, tag="lnv")
            nc.scalar.activation(out=lnv, in_=ssq,
                                 func=mybir.ActivationFunctionType.Ln,
                                 bias=sb_eps, scale=1.0 / D)
            rstd = scal.tile([P, 1], f32, tag="rstd")
            nc.scalar.activation(out=rstd, in_=lnv,
                                 func=mybir.ActivationFunctionType.Exp,
                                 scale=-0.5)
            rt = pool_sb.tile([P, D], bf16, tag="rt")
            if not gen_ln:
                # Relu((pre)*rstd + (-mu*rstd)) in one ACT op
                nmr = scal.tile([P, 1], f32, tag="nmr")
                nc.vector.tensor_scalar(
                    out=nmr, in0=mu, scalar1=rstd, scalar2=-1.0,
                    op0=mybir.AluOpType.mult, op1=mybir.AluOpType.mult,
                )
                nc.scalar.activation(out=rt, in_=pre[:, 0:D],
                                     func=mybir.ActivationFunctionType.Relu,
                                     bias=nmr, scale=rstd)
            else:
                tt = pool_sb.tile([P, D], f32, tag="tt")
                nc.vector.tensor_scalar(
                    out=tt, in0=pre[:, 0:D], scalar1=mu, scalar2=rstd,
                    op0=mybir.AluOpType.subtract, op1=mybir.AluOpType.mult,
                )
                if has_gamma:
                    nc.vector.tensor_mul(out=tt, in0=tt, in1=gam_bc)
                if has_beta:
                    nc.vector.tensor_add(out=tt, in0=tt, in1=bet_bc)
                nc.vector.tensor_scalar_max(out=rt, in0=tt, scalar1=0.0)
            nc.tensor.matmul(ps_r, lhsT=rt, rhs=ones_b,
                             start=first, stop=last)

        ngrp = MMN // P
        with tc.tile_pool(name="psS2", bufs=1, space="PSUM") as psS2, \
             tc.tile_pool(name="psD", bufs=1, space="PSUM") as psD, \
             tc.tile_pool(name="psP", bufs=2, space="PSUM") as psP, \
             tc.tile_pool(name="psR", bufs=1, space="PSUM") as psR:
            ps_s2t = psS2.tile([P, M], f32, tag="psS2")
            ps_r2 = psR.tile([P, 1], f32, tag="psr")
            for c in range(M // MMN):
                ps_den = psD.tile([1, MMN], f32, tag="psD")
                for t in range(NT):
                    nc.tensor.matmul(
                        ps_den, lhsT=ones_b, rhs=sb_E1[:, t, bass.ts(c, MMN)],
                        start=(t == 0), stop=(t == NT - 1),
                    )
                row = sb_d2row[:, bass.ts(c, MMN)]
                nc.vector.tensor_copy(out=row, in_=ps_den)
                for tt_ in range(ngrp):
                    tcol = c * ngrp + tt_
                    nc.gpsimd.dma_start(
                        out=den2p[:, tcol : tcol + 1],
                        in_=sb_d2row[:, bass.ds(tcol * P, P)],
                    )
                nc.vector.reciprocal(out=rec2[:, bass.ts(c, ngrp)],
                                     in_=den2p[:, bass.ts(c, ngrp)])
                # S2T contributions for i-tiles of this chunk
                for k in range(ngrp):
                    ci = c * ngrp + k
                    hp = fpp.tile([P, D], bf16, tag="hp")
                    nc.scalar.activation(out=hp, in_=sb_Hb[:, bass.ts(ci, P)],
                                         func=mybir.ActivationFunctionType.Copy,
                                         scale=rec2[:, ci : ci + 1])
                    for h in range(M // MMN):
                        hs = bass.ts(h, MMN)
                        nc.tensor.matmul(
                            ps_s2t[:, hs], lhsT=hp, rhs=sb_E2[:, ci, hs],
                            start=(ci == 0), stop=(ci == NT - 1),
                        )
                # MLP2 tiles interleaved per chunk
                for k in range(ngrp):
                    t_ = c * ngrp + k
                    mlp_tile(t_, sb_HTb, sb_S1Tb, ps_r2,
                             first=(t_ == 0), last=(t_ == NT - 1), pool_ps=psP,
                             pool_sb=mlpt)
            r2_sb = consts.tile([P, 1], f32)
            nc.vector.tensor_copy(out=r2_sb, in_=ps_r2)

            # S2T evac
            nc.vector.tensor_copy(out=sb_S2Tb[:, 0:CH], in_=ps_s2t[:, 0:CH])
            nc.scalar.copy(out=sb_S2Tb[:, CH:M], in_=ps_s2t[:, CH:M])

            # ============= P3: MLP1 + pooled vectors + final ================
            ps_r1 = psR.tile([P, 1], f32, tag="psr")
            for t_ in range(NT):
                mlp_tile(t_, sb_FTb, sb_S2Tb, ps_r1,
                         first=(t_ == 0), last=(t_ == NT - 1), pool_ps=psP,
                         pool_sb=mlpt)
            r1_sb = consts.tile([P, 1], f32)
            nc.vector.tensor_copy(out=r1_sb, in_=ps_r1)

            pcat = consts.tile([P, 2], f32)

            def pvec(r_sb, lcol, out_slice):
                ps_p = psP.tile([P, 1], f32, tag="pre")
                nc.tensor.matmul(ps_p, lhsT=sb_W2, rhs=r_sb, start=True,
                                 stop=not has_b2)
                if has_b2:
                    nc.tensor.matmul(ps_p, lhsT=sb_B2, rhs=c2048, start=False,
                                     stop=True)
                nc.vector.tensor_add(out=out_slice, in0=ps_p,
                                     in1=sb_LV[:, lcol : lcol + 1])

            pvec(r2_sb, 1, pcat[:, 1:2])
            pvec(r1_sb, 0, pcat[:, 0:1])

            # out = s12 / sqrt(s11*s22) via exp(-0.5*ln(.))
            ps_d1 = psP.tile([1, 2], f32, tag="pre")
            nc.tensor.matmul(ps_d1, lhsT=pcat[:, 0:1], rhs=pcat, start=True,
                             stop=True)
            ps_d2 = psP.tile([1, 1], f32, tag="pre")
            nc.tensor.matmul(ps_d2, lhsT=pcat[:, 1:2], rhs=pcat[:, 1:2],
                             start=True, stop=True)
            dots = consts.tile([1, 4], f32)
            nc.vector.tensor_copy(out=dots[:, 0:2], in_=ps_d1)   # s11, s12
            nc.vector.tensor_copy(out=dots[:, 2:3], in_=ps_d2)   # s22
            q = consts.tile([1, 1], f32)
            nc.vector.tensor_mul(out=q, in0=dots[:, 0:1], in1=dots[:, 2:3])
            nc.vector.tensor_scalar_max(out=q, in0=q, scalar1=1e-30)
            lq = consts.tile([1, 1], f32)
            nc.scalar.activation(out=lq, in_=q,
                                 func=mybir.ActivationFunctionType.Ln)
            rq = consts.tile([1, 1], f32)
            nc.scalar.activation(out=rq, in_=lq,
                                 func=mybir.ActivationFunctionType.Exp,
                                 scale=-0.5)
            res = consts.tile([1, 1], f32)
            nc.vector.tensor_mul(out=res, in0=dots[:, 1:2], in1=rq)
            nc.sync.dma_start(out=dOUT[:, :], in_=res)

    split_waits(nc)
    return nc


_BUILD_CACHE = {}


def _get_nc(flags):
    if flags not in _BUILD_CACHE:
        _BUILD_CACHE[flags] = build_nc(*flags)
    return _BUILD_CACHE[flags]


def kernel(x, edge_attr, W1, b1, gamma, beta, W2, b2, gid, edge_index, batch):
    import ml_dtypes

    nbf16 = ml_dtypes.bfloat16
    x = np.asarray(x, dtype=np.float32)
    W1 = np.asarray(W1, dtype=np.float32)
    b1 = np.asarray(b1, dtype=np.float32)
    gamma = np.asarray(gamma, dtype=np.float32)
    beta = np.asarray(beta, dtype=np.float32)
    W2 = np.asarray(W2, dtype=np.float32)
    b2 = np.asarray(b2, dtype=np.float32)
    gid = int(np.asarray(gid))
    ei0 = np.asarray(edge_index)[0]
    b = np.asarray(batch)

    N, Dx = x.shape
    assert Dx == D
    deg = np.bincount(ei0, minlength=N)
    mask = deg > 1
    G = int(b.max()) + 1
    assert G == 8
    hd_idx = np.where(mask)[0]
    fhb = b[hd_idx]
    Mtot = hd_idx.size
    assert Mtot % G == 0 and np.array_equal(
        fhb, np.repeat(np.arange(G), Mtot // G)
    )
    assert Mtot // G == M

    gxf_idx = np.where(mask & (b == gid))[0]
    assert gxf_idx.size == M
    F = np.ascontiguousarray(x[gxf_idx])
    FTb = np.ascontiguousarray(F.T).astype(nbf16)
    # Fb[p, t*128:(t+1)*128] = F[t*128+p, :]  (per-tile natural rows)
    Fb = np.ascontiguousarray(
        F.reshape(NT, P, D).transpose(1, 0, 2).reshape(P, M)
    ).astype(nbf16)
    lo0_idx = np.where((~mask) & (b == gid))[0]
    assert lo0_idx.size == M
    l1 = x[lo0_idx].astype(np.float64).sum(0).astype(np.float32)

    # per-core scalar exp shift: gid core centers the window on the row-norm
    # range (diagonal dominates there); others use a constant
    sq = (F.astype(np.float64) ** 2).sum(1)
    c_gid = float((sq.max() + sq.min()) / 2.0)

    flags = (
        bool(np.any(b1 != 0.0)),
        bool(np.any(b2 != 0.0)),
        bool(np.any(gamma != 1.0)),
        bool(np.any(beta != 0.0)),
    )
    has_b1, has_b2, has_gamma, has_beta = flags
    nc = _get_nc(flags)

    # W1 augmented: [128, 2, 132]; col 128 = W1half @ 1/128
    WA = 132
    W1a = np.zeros((P, 2, WA), np.float32)
    W1r = W1.reshape(2, P, D)
    W1a[:, :, 0:D] = W1r.transpose(1, 0, 2)
    W1a[:, :, D] = (W1r.sum(2) / D).T  # [P, 2]
    W1ab = W1a.astype(nbf16)

    in_maps = []
    for g in range(G):
        sel_h = mask & (b == g)
        sel_l = (~mask) & (b == g)
        assert sel_h.sum() == M and sel_l.sum() == M
        H = np.ascontiguousarray(x[sel_h])
        l2 = x[sel_l].astype(np.float64).sum(0).astype(np.float32)
        LV = np.stack([l1, l2], axis=1)  # [128, 2]
        cshift = c_gid if g == gid else SHIFT0
        im = {
            "HTb": np.ascontiguousarray(H.T).astype(nbf16),
            "FTb": FTb,
            "Hb": np.ascontiguousarray(
                H.reshape(NT, P, D).transpose(1, 0, 2).reshape(P, M)
            ).astype(nbf16),
            "Fb": Fb,
            "W1a": W1ab,
            "W2": W2,
            "NEG": np.full((P, 1), -cshift, np.float32),
            "LV": LV,
        }
        if has_b1:
            b1a = np.zeros((1, WA), np.float32)
            b1a[0, 0:D] = b1
            b1a[0, D] = b1.sum() / D
            im["B1a"] = b1a.astype(nbf16)
        if has_b2:
            im["B2"] = b2.reshape(1, D).astype(np.float32)
        if has_gamma:
            im["GAM"] = gamma.reshape(1, D).astype(np.float32)
        if has_beta:
            im["BET"] = beta.reshape(1, D).astype(np.float32)
        in_maps.append(im)

    trace_dir = os.environ.get("ADAGMN_TRACE", "")
    if trace_dir:
        res = run_bass_kernel_spmd(
            nc, in_maps, core_ids=list(range(G)), trace=True, tmpdir=trace_dir
        )
        print(f"HW exec time: {res.exec_time_ns} ns")
    else:
        res = run_bass_kernel_spmd(nc, in_maps, core_ids=list(range(G)))
    out = np.array([res.results[g]["out"][0, 0] for g in range(G)], np.float32)
    return out


# revision 16
# speedup vs baseline: 1.3537x; 1.3537x over previous
"""Trainium2 Bass kernel for nn_AdaGMNConv (gnn_message_passing).

Sharding: one graph per NeuronCore (G=8 graphs, 8 cores). All compute is
local to a core; the host gathers the per-graph scalar outputs.

Per-core math (graph g, M=2048 high-degree nodes per graph, D=128):
  A    = H_g @ F^T                      [2048, 2048]   (bf16 matmul, f32 psum)
  A1   = segment softmax of A over rows (per column)   -> S1 = A1 @ F
  A2   = softmax of A over columns (per row)           -> S2 = A2^T @ H
  out_multi  = MLP([H | S1]); out_single = MLP([F | S2])
  p2 = colsum(out_multi) + colsum(L_g);  p1 = colsum(out_single) + colsum(L_gid)
  out[g] = <p1/||p1||, p2/||p2||>

Schedule (single exp pass; E^T via DMA transpose):
  P1 (per j-tile t): A^T-chunk MMs -> fused exp(+row-sum accum = den1)
     -> F~ = F/den1 -> S1T accumulation MMs; DMA-transpose E1_t into E2.
  P2: S1T evac; den2 = col-sums of E1 via ones-matmuls (chunked, PSUM
     rotating), scatter to per-partition layout, H~ = H/den2 -> S2T MMs;
     MLP(out_multi) interleaved on ACT/DVE.
  P3: S2T evac; MLP(out_single); pooled vectors; cosine output.

MLP tricks: W1 is host-augmented with a 129th column = W1 @ 1/128 so the
row-mean of the pre-activation falls out of the matmul; LayerNorm+ReLU is
one ScalarE op (scale=rstd, bias=-mu*rstd per partition); the column sum
of the ReLU output accumulates through 1-wide matmuls. L/L0 column sums
are host-precomputed (they are linear in x).
"""

import os
from contextlib import ExitStack

import numpy as np

import concourse.bass as bass
import concourse.tile as tile
from concourse import mybir
from concourse.bass_utils import run_bass_kernel_spmd

f32 = mybir.dt.float32
bf16 = mybir.dt.bfloat16

P = 128          # partitions
D = 128          # feature dim
NT = 16          # tiles per 2048-node block
M = P * NT       # 2048 nodes per block
SHIFT0 = 64.0    # exp shift for non-gid cores
LN_EPS = 1e-5
CH = 1024        # PSUM chunk width for the A tiles (2 banks)
MMN = 512        # matmul moving free-dim max

MAXW = 1  # walrus in this env rejects >1 sem-wait per instruction


def split_waits(nc, maxw=MAXW):
    """Hoist overflow sem-waits onto preceding same-engine NOPs (this walrus
    build only accepts `maxw` waits per instruction)."""
    ctr = 0
    for fn in nc.m.functions:
        for bb in fn.blocks:
            new_insts = []
            for inst in bb.instructions:
                si = inst.sync_info
                if si is not None and si.on_wait and len(si.on_wait) > maxw:
                    waits = list(si.on_wait)
                    chunks = [waits[i : i + maxw] for i in range(0, len(waits), maxw)]
                    for ch in chunks[:-1]:
                        ctr += 1
                        nop = mybir.InstNoOp(
                            name=f"waitsplit_{ctr}",
                            sync_info=mybir.SyncInfo(on_wait=ch, on_update=[]),
                            bass_nofuse=True,
                            engine=inst.engine,
                        )
                        new_insts.append(nop)
                    si.on_wait = chunks[-1]
                new_insts.append(inst)
            bb.instructions = new_insts
    return ctr


def build_nc(has_b1, has_b2, has_gamma, has_beta):
    nc = bass.Bass()
    WA = 132  # padded aug width (129 used)

    # ---- DRAM parameters (per-core shard shapes, host-packed layouts) ----
    dHTb = nc.declare_dram_parameter("HTb", [P, M], bf16, isOutput=False)
    dFTb = nc.declare_dram_parameter("FTb", [P, M], bf16, isOutput=False)
    dHb = nc.declare_dram_parameter("Hb", [P, M], bf16, isOutput=False)
    dFb = nc.declare_dram_parameter("Fb", [P, M], bf16, isOutput=False)
    dW1 = nc.declare_dram_parameter("W1a", [P, 2, WA], bf16, isOutput=False)
    dW2 = nc.declare_dram_parameter("W2", [P, D], f32, isOutput=False)
    dNEG = nc.declare_dram_parameter("NEG", [P, 1], f32, isOutput=False)
    dLV = nc.declare_dram_parameter("LV", [P, 2], f32, isOutput=False)
    dB1 = dB2 = dGAM = dBET = None
    if has_b1:
        dB1 = nc.declare_dram_parameter("B1a", [1, WA], bf16, isOutput=False)
    if has_b2:
        dB2 = nc.declare_dram_parameter("B2", [1, D], f32, isOutput=False)
    if has_gamma:
        dGAM = nc.declare_dram_parameter("GAM", [1, D], f32, isOutput=False)
    if has_beta:
        dBET = nc.declare_dram_parameter("BET", [1, D], f32, isOutput=False)
    dOUT = nc.declare_dram_parameter("out", [1, 1], f32, isOutput=True)

    gen_ln = has_gamma or has_beta  # generic LN path on DVE

    with tile.TileContext(nc) as tc, ExitStack() as ctx:
        consts = ctx.enter_context(tc.tile_pool(name="consts", bufs=1))
        scal = ctx.enter_context(tc.tile_pool(name="scal", bufs=6))
        fpp = ctx.enter_context(tc.tile_pool(name="fpp", bufs=3))
        mlpt = ctx.enter_context(tc.tile_pool(name="mlpt", bufs=3))

        # ---- SBUF input loads (fat contiguous DMAs, host-packed) ----
        sb_FTb = consts.tile([P, M], bf16)
        nc.sync.dma_start(out=sb_FTb, in_=dFTb[:, :])
        sb_HTb = consts.tile([P, M], bf16)
        for c in range(2):
            nc.sync.dma_start(out=sb_HTb[:, bass.ts(c, M // 2)],
                              in_=dHTb[:, bass.ts(c, M // 2)])
        sb_NEG = consts.tile([P, 1], f32)
        nc.gpsimd.dma_start(out=sb_NEG, in_=dNEG[:, :])
        sb_Fb = consts.tile([P, M], bf16)
        nc.gpsimd.dma_start(out=sb_Fb, in_=dFb[:, :])
        sb_Hb = consts.tile([P, M], bf16)
        nc.gpsimd.dma_start(out=sb_Hb, in_=dHb[:, :])
        sb_W1 = consts.tile([P, 2, WA], bf16)
        nc.gpsimd.dma_start(out=sb_W1, in_=dW1[:, :, :])
        sb_W2 = consts.tile([P, D], f32)
        nc.gpsimd.dma_start(out=sb_W2, in_=dW2[:, :])
        sb_LV = consts.tile([P, 2], f32)
        nc.gpsimd.dma_start(out=sb_LV, in_=dLV[:, :])

        sb_B1 = sb_B2 = None
        if has_b1:
            sb_B1 = consts.tile([1, WA], bf16)
            nc.gpsimd.dma_start(out=sb_B1, in_=dB1[:, :])
        if has_b2:
            sb_B2 = consts.tile([1, D], f32)
            nc.gpsimd.dma_start(out=sb_B2, in_=dB2[:, :])
        gam_bc = bet_bc = None
        if has_gamma:
            gam_bc = consts.tile([P, D], f32)
            src = dGAM[:, :]
            nc.gpsimd.dma_start(
                out=gam_bc,
                in_=bass.AP(tensor=src.tensor, offset=src.offset,
                            ap=[[0, P], src.ap[1]]),
            )
        if has_beta:
            bet_bc = consts.tile([P, D], f32)
            src = dBET[:, :]
            nc.gpsimd.dma_start(
                out=bet_bc,
                in_=bass.AP(tensor=src.tensor, offset=src.offset,
                            ap=[[0, P], src.ap[1]]),
            )

        ones_f = consts.tile([P, 1], f32)
        nc.vector.memset(ones_f, 1.0)
        ones_b = consts.tile([P, 1], bf16)
        nc.vector.memset(ones_b, 1.0)
        ones_row = consts.tile([1, D], bf16)
        nc.vector.memset(ones_row, 1.0)
        sb_eps = consts.tile([P, 1], f32)
        nc.vector.memset(sb_eps, LN_EPS)
        c2048 = consts.tile([1, 1], f32)
        nc.vector.memset(c2048, float(M))

        sb_E1 = consts.tile([P, NT, M], bf16)
        sb_E2 = consts.tile([P, NT, M], bf16)
        sb_S1Tb = consts.tile([P, M], bf16)
        sb_S2Tb = consts.tile([P, M], bf16)
        den2p = consts.tile([P, NT], f32)
        rec2 = consts.tile([P, NT], f32)
        sb_d2row = consts.tile([1, M], f32)

        # =========== P1: A^T tiles -> exp/den1 -> F~ -> S1T; transpose ======
        with tc.tile_pool(name="psS", bufs=1, space="PSUM") as psS, \
             tc.tile_pool(name="psA", bufs=2, space="PSUM") as psA:
            ps_s1t = psS.tile([P, M], f32, tag="psS")
            for t in range(NT):
                tr = bass.ts(t, P)
                dp = scal.tile([P, 2], f32, tag="dp")
                for c in range(M // CH):
                    pa = psA.tile([P, CH], f32, tag="psA")
                    for h in range(CH // MMN):
                        ci = c * (CH // MMN) + h
                        nc.tensor.matmul(
                            pa[:, bass.ts(h, MMN)], lhsT=sb_FTb[:, tr],
                            rhs=sb_HTb[:, bass.ts(ci, MMN)],
                            start=True, stop=True,
                        )
                    nc.scalar.activation(
                        out=sb_E1[:, t, bass.ts(c, CH)], in_=pa,
                        func=mybir.ActivationFunctionType.Exp,
                        bias=sb_NEG, scale=1.0,
                        accum_out=dp[:, c : c + 1],
                    )
                den = scal.tile([P, 1], f32, tag="den")
                nc.vector.reduce_sum(out=den, in_=dp, axis=mybir.AxisListType.X)
                rec = scal.tile([P, 1], f32, tag="rec")
                nc.vector.reciprocal(out=rec, in_=den)
                fp = fpp.tile([P, D], bf16, tag="fp")
                nc.vector.tensor_scalar_mul(out=fp, in0=sb_Fb[:, tr], scalar1=rec)
                for h in range(M // MMN):
                    hs = bass.ts(h, MMN)
                    nc.tensor.matmul(
                        ps_s1t[:, hs], lhsT=fp, rhs=sb_E1[:, t, hs],
                        start=(t == 0), stop=(t == NT - 1),
                    )
                nc.sync.dma_start_transpose(out=sb_E2[:, :, tr], in_=sb_E1[:, t, :])

            # S1T evac (split DVE/ACT halves to spread the cost)
            nc.vector.tensor_copy(out=sb_S1Tb[:, 0:CH], in_=ps_s1t[:, 0:CH])
            nc.scalar.copy(out=sb_S1Tb[:, CH:M], in_=ps_s1t[:, CH:M])

        # ============= P2: den2 (ones-MMs) -> H~ -> S2T; MLP2 ==============
        # MLP formulation: ReLU((pre-mu)*rstd) = rstd*ReLU(pre-mu), so the
        # per-tile work is bn_stats/bn_aggr + one fused (sub,max) DVE op; the
        # 16 rstds are computed in ONE batched Ln+Exp pair, and the per-row
        # rstd scaling folds into the 1-wide colsum matmul as its rhs.
        sb_rt = consts.tile([P, NT, D], bf16)
        mv_all = consts.tile([P, NT, 2], f32)
        rstd_b = consts.tile([P, NT], bf16)

        def mlp_stats_tile(t, xTb, sTb, pool_ps):
            """Pre-act MM + stats + relu(pre-mu) for one tile."""
            tr = bass.ts(t, P)
            pre = pool_ps.tile([P, D], f32, tag="pre")
            nc.tensor.matmul(pre, lhsT=xTb[:, tr], rhs=sb_W1[:, 0, 0:D],
                             start=True, stop=False)
            nc.tensor.matmul(pre, lhsT=sTb[:, tr], rhs=sb_W1[:, 1, 0:D],
                             start=False, stop=not has_b1)
            if has_b1:
                nc.tensor.matmul(pre, lhsT=ones_row, rhs=sb_B1[:, 0:D],
                                 start=False, stop=True)
            stats = scal.tile([P, 6], f32, tag="stats")
            nc.vector.bn_stats(out=stats, in_=pre)
            nc.vector.bn_aggr(out=mv_all[:, t, :], in_=stats)
            if not gen_ln:
                # ReLU(pre - mu); the rstd scale folds into the colsum MM
                nc.vector.tensor_scalar(
                    out=sb_rt[:, t, :], in0=pre, scalar1=mv_all[:, t, 0:1],
                    scalar2=0.0,
                    op0=mybir.AluOpType.subtract, op1=mybir.AluOpType.max,
                )
            else:
                # store (pre - mu); normalize/affine/relu happen in finish
                nc.vector.tensor_scalar(
                    out=sb_rt[:, t, :], in0=pre, scalar1=mv_all[:, t, 0:1],
                    scalar2=1.0,
                    op0=mybir.AluOpType.subtract, op1=mybir.AluOpType.mult,
                )

        def mlp_finish(ps_r):
            """Batched rstd + 16 colsum-MMs (rhs=rstd when foldable)."""
            lnv = scal.tile([P, NT], f32, tag="lnv")
            nc.scalar.activation(out=lnv, in_=mv_all[:, :, 1],
                                 func=mybir.ActivationFunctionType.Ln,
                                 bias=sb_eps, scale=1.0)
            rstd = scal.tile([P, NT], f32, tag="rstd")
            nc.scalar.activation(out=rstd, in_=lnv,
                                 func=mybir.ActivationFunctionType.Exp,
                                 scale=-0.5)
            if not gen_ln:
                nc.vector.tensor_copy(out=rstd_b, in_=rstd)
                for t in range(NT):
                    nc.tensor.matmul(ps_r, lhsT=sb_rt[:, t, :],
                                     rhs=rstd_b[:, t : t + 1],
                                     start=(t == 0), stop=(t == NT - 1))
            else:
                for t in range(NT):
                    tt = mlpt.tile([P, D], f32, tag="tt")
                    nc.vector.tensor_scalar_mul(out=tt, in0=sb_rt[:, t, :],
                                                scalar1=rstd[:, t : t + 1])
                    if has_gamma:
                        nc.vector.tensor_mul(out=tt, in0=tt, in1=gam_bc)
                    if has_beta:
                        nc.vector.tensor_add(out=tt, in0=tt, in1=bet_bc)
                    rt = mlpt.tile([P, D], bf16, tag="rt")
                    nc.vector.tensor_scalar_max(out=rt, in0=tt, scalar1=0.0)
                    nc.tensor.matmul(ps_r, lhsT=rt, rhs=ones_b,
                                     start=(t == 0), stop=(t == NT - 1))

        ngrp = MMN // P
        with tc.tile_pool(name="psS2", bufs=1, space="PSUM") as psS2, \
             tc.tile_pool(name="psD", bufs=1, space="PSUM") as psD, \
             tc.tile_pool(name="psP", bufs=2, space="PSUM") as psP, \
             tc.tile_pool(name="psR", bufs=1, space="PSUM") as psR:
            ps_s2t = psS2.tile([P, M], f32, tag="psS2")
            ps_r2 = psR.tile([P, 1], f32, tag="psr")
            for c in range(M // MMN):
                ps_den = psD.tile([1, MMN], f32, tag="psD")
                for t in range(NT):
                    nc.tensor.matmul(
                        ps_den, lhsT=ones_b, rhs=sb_E1[:, t, bass.ts(c, MMN)],
                        start=(t == 0), stop=(t == NT - 1),
                    )
                row = sb_d2row[:, bass.ts(c, MMN)]
                nc.vector.tensor_copy(out=row, in_=ps_den)
                for tt_ in range(ngrp):
                    tcol = c * ngrp + tt_
                    nc.gpsimd.dma_start(
                        out=den2p[:, tcol : tcol + 1],
                        in_=sb_d2row[:, bass.ds(tcol * P, P)],
                    )
                nc.vector.reciprocal(out=rec2[:, bass.ts(c, ngrp)],
                                     in_=den2p[:, bass.ts(c, ngrp)])
                # S2T contributions for i-tiles of this chunk
                for k in range(ngrp):
                    ci = c * ngrp + k
                    hp = fpp.tile([P, D], bf16, tag="hp")
                    nc.vector.tensor_scalar_mul(out=hp, in0=sb_Hb[:, bass.ts(ci, P)],
                                                scalar1=rec2[:, ci : ci + 1])
                    for h in range(M // MMN):
                        hs = bass.ts(h, MMN)
                        nc.tensor.matmul(
                            ps_s2t[:, hs], lhsT=hp, rhs=sb_E2[:, ci, hs],
                            start=(ci == 0), stop=(ci == NT - 1),
                        )
                # MLP2 stats tiles interleaved per chunk
                for k in range(ngrp):
                    t_ = c * ngrp + k
                    mlp_stats_tile(t_, sb_HTb, sb_S1Tb, psP)
            mlp_finish(ps_r2)
            r2_sb = consts.tile([P, 1], f32)
            nc.vector.tensor_copy(out=r2_sb, in_=ps_r2)

            # S2T evac
            nc.vector.tensor_copy(out=sb_S2Tb[:, 0:CH], in_=ps_s2t[:, 0:CH])
            nc.scalar.copy(out=sb_S2Tb[:, CH:M], in_=ps_s2t[:, CH:M])

            # ============= P3: MLP1 + pooled vectors + final ================
            ps_r1 = psR.tile([P, 1], f32, tag="psr")
            for t_ in range(NT):
                mlp_stats_tile(t_, sb_FTb, sb_S2Tb, psP)
            mlp_finish(ps_r1)
            r1_sb = consts.tile([P, 1], f32)
            nc.vector.tensor_copy(out=r1_sb, in_=ps_r1)

            pcat = consts.tile([P, 2], f32)

            def pvec(r_sb, lcol, out_slice):
                ps_p = psP.tile([P, 1], f32, tag="pre")
                nc.tensor.matmul(ps_p, lhsT=sb_W2, rhs=r_sb, start=True,
                                 stop=not has_b2)
                if has_b2:
                    nc.tensor.matmul(ps_p, lhsT=sb_B2, rhs=c2048, start=False,
                                     stop=True)
                nc.vector.tensor_add(out=out_slice, in0=ps_p,
                                     in1=sb_LV[:, lcol : lcol + 1])

            pvec(r2_sb, 1, pcat[:, 1:2])
            pvec(r1_sb, 0, pcat[:, 0:1])

            # out = s12 / sqrt(s11*s22) via exp(-0.5*ln(.))
            ps_d1 = psP.tile([1, 2], f32, tag="pre")
            nc.tensor.matmul(ps_d1, lhsT=pcat[:, 0:1], rhs=pcat, start=True,
                             stop=True)
            ps_d2 = psP.tile([1, 1], f32, tag="pre")
            nc.tensor.matmul(ps_d2, lhsT=pcat[:, 1:2], rhs=pcat[:, 1:2],
                             start=True, stop=True)
            dots = consts.tile([1, 4], f32)
            nc.vector.tensor_copy(out=dots[:, 0:2], in_=ps_d1)   # s11, s12
            nc.vector.tensor_copy(out=dots[:, 2:3], in_=ps_d2)   # s22
            q = consts.tile([1, 1], f32)
            nc.vector.tensor_mul(out=q, in0=dots[:, 0:1], in1=dots[:, 2:3])
            nc.vector.tensor_scalar_max(out=q, in0=q, scalar1=1e-30)
            lq = consts.tile([1, 1], f32)
            nc.scalar.activation(out=lq, in_=q,
                                 func=mybir.ActivationFunctionType.Ln)
            rq = consts.tile([1, 1], f32)
            nc.scalar.activation(out=rq, in_=lq,
                                 func=mybir.ActivationFunctionType.Exp,
                                 scale=-0.5)
            res = consts.tile([1, 1], f32)
            nc.vector.tensor_mul(out=res, in0=dots[:, 1:2], in1=rq)
            nc.sync.dma_start(out=dOUT[:, :], in_=res)

    split_waits(nc)
    return nc


_BUILD_CACHE = {}


def _get_nc(flags):
    if flags not in _BUILD_CACHE:
        _BUILD_CACHE[flags] = build_nc(*flags)
    return _BUILD_CACHE[flags]


def kernel(x, edge_attr, W1, b1, gamma, beta, W2, b2, gid, edge_index, batch):
    import ml_dtypes

    nbf16 = ml_dtypes.bfloat16
    x = np.asarray(x, dtype=np.float32)
    W1 = np.asarray(W1, dtype=np.float32)
    b1 = np.asarray(b1, dtype=np.float32)
    gamma = np.asarray(gamma, dtype=np.float32)
    beta = np.asarray(beta, dtype=np.float32)
    W2 = np.asarray(W2, dtype=np.float32)
    b2 = np.asarray(b2, dtype=np.float32)
    gid = int(np.asarray(gid))
    ei0 = np.asarray(edge_index)[0]
    b = np.asarray(batch)

    N, Dx = x.shape
    assert Dx == D
    deg = np.bincount(ei0, minlength=N)
    mask = deg > 1
    G = int(b.max()) + 1
    assert G == 8
    hd_idx = np.where(mask)[0]
    fhb = b[hd_idx]
    Mtot = hd_idx.size
    assert Mtot % G == 0 and np.array_equal(
        fhb, np.repeat(np.arange(G), Mtot // G)
    )
    assert Mtot // G == M

    gxf_idx = np.where(mask & (b == gid))[0]
    assert gxf_idx.size == M
    F = np.ascontiguousarray(x[gxf_idx])
    FTb = np.ascontiguousarray(F.T).astype(nbf16)
    # Fb[p, t*128:(t+1)*128] = F[t*128+p, :]  (per-tile natural rows)
    Fb = np.ascontiguousarray(
        F.reshape(NT, P, D).transpose(1, 0, 2).reshape(P, M)
    ).astype(nbf16)
    lo0_idx = np.where((~mask) & (b == gid))[0]
    assert lo0_idx.size == M
    l1 = x[lo0_idx].astype(np.float64).sum(0).astype(np.float32)

    # per-core scalar exp shift: gid core centers the window on the row-norm
    # range (diagonal dominates there); others use a constant
    sq = (F.astype(np.float64) ** 2).sum(1)
    c_gid = float((sq.max() + sq.min()) / 2.0)

    flags = (
        bool(np.any(b1 != 0.0)),
        bool(np.any(b2 != 0.0)),
        bool(np.any(gamma != 1.0)),
        bool(np.any(beta != 0.0)),
    )
    has_b1, has_b2, has_gamma, has_beta = flags
    nc = _get_nc(flags)

    # W1 augmented: [128, 2, 132]; col 128 = W1half @ 1/128
    WA = 132
    W1a = np.zeros((P, 2, WA), np.float32)
    W1r = W1.reshape(2, P, D)
    W1a[:, :, 0:D] = W1r.transpose(1, 0, 2)
    W1a[:, :, D] = (W1r.sum(2) / D).T  # [P, 2]
    W1ab = W1a.astype(nbf16)

    in_maps = []
    for g in range(G):
        sel_h = mask & (b == g)
        sel_l = (~mask) & (b == g)
        assert sel_h.sum() == M and sel_l.sum() == M
        H = np.ascontiguousarray(x[sel_h])
        l2 = x[sel_l].astype(np.float64).sum(0).astype(np.float32)
        LV = np.stack([l1, l2], axis=1)  # [128, 2]
        cshift = c_gid if g == gid else SHIFT0
        im = {
            "HTb": np.ascontiguousarray(H.T).astype(nbf16),
            "FTb": FTb,
            "Hb": np.ascontiguousarray(
                H.reshape(NT, P, D).transpose(1, 0, 2).reshape(P, M)
            ).astype(nbf16),
            "Fb": Fb,
            "W1a": W1ab,
            "W2": W2,
            "NEG": np.full((P, 1), -cshift, np.float32),
            "LV": LV,
        }
        if has_b1:
            b1a = np.zeros((1, WA), np.float32)
            b1a[0, 0:D] = b1
            b1a[0, D] = b1.sum() / D
            im["B1a"] = b1a.astype(nbf16)
        if has_b2:
            im["B2"] = b2.reshape(1, D).astype(np.float32)
        if has_gamma:
            im["GAM"] = gamma.reshape(1, D).astype(np.float32)
        if has_beta:
            im["BET"] = beta.reshape(1, D).astype(np.float32)
        in_maps.append(im)

    trace_dir = os.environ.get("ADAGMN_TRACE", "")
    if trace_dir:
        res = run_bass_kernel_spmd(
            nc, in_maps, core_ids=list(range(G)), trace=True, tmpdir=trace_dir
        )
        print(f"HW exec time: {res.exec_time_ns} ns")
    else:
        res = run_bass_kernel_spmd(nc, in_maps, core_ids=list(range(G)))
    out = np.array([res.results[g]["out"][0, 0] for g in range(G)], np.float32)
    return out
